# revision 52
# baseline (speedup 1.0000x reference)
"""Causal self-attention (B=8, T=1024, C=2048, H=16) on 8 TRN2 NeuronCores.

Strategy: data-parallel over batch — core i computes the full attention block
for batch element i (weights replicated, no collectives).

Per-core pipeline (Tile framework, all matmuls bf16 on the PE):
  A) x [T,C] f32 -> ACT cast to bf16 -> PE-transpose (bf16, 1cyc/row) into
     xT halves (xTa = rows 0..511, xTb = rows 512..1023) so phase B's qt=0
     matmuls can start once half of x has landed.
  B) qkv^T = W-chunk-stationary matmuls vs xT moving; PSUM->SBUF copies fuse
     bias (+ softmax scale for q) and cast to bf16. The first 4 v-chunks run
     their qt=0 sweep early (interleaved with the tail of phase A) to keep the
     PE busy during the x DMA. v-chunks are produced transposed and
     PE-transposed back to natural [T, C] layout.
  C) per head: S^T blocks (kT-chunk stationary, qT moving), pair-packed into
     [128,1024] PSUM tiles so one ACT exp covers two blocks; causal masking of
     diagonal blocks via precomputed 0/1 masks on DVE; eS block PAIRS summed on
     the (otherwise idle) GPSIMD engine so the ones-matmul denominators need
     half the PE streaming; PV accumulates out^T; divide by denom on DVE.
  D) y = attnT-stationary @ w_proj, ct-outer with the full [C,512] w_proj
     slice resident in SBUF (bf16) and two 4-bank PSUM t-half groups rotating
     so ct boundaries overlap; bias added on DVE from a host-precomputed
     broadcast tile during the PSUM->SBUF copy; output f32.
"""

import sys

if "/opt/trn_rl_repo" not in sys.path:
    sys.path.insert(0, "/opt/trn_rl_repo")

import numpy as np
import ml_dtypes

import concourse.bass as bass
import concourse.mybir as mybir
import concourse.tile as tile
from concourse import bacc
from concourse.bass_utils import run_bass_kernel_spmd

B, T, C = 8, 1024, 2048
H, HD = 16, 128
N_CORES = 8
P = 128            # partition dim
TQ = 512           # moving-operand tile (q positions per matmul)
KK = C // P        # 16 contraction tiles over C
TT = T // P        # 8 tiles over T
NQ = T // TQ       # 2 q-tiles
SCALE = 1.0 / float(np.sqrt(HD))

f32 = mybir.dt.float32
bf16 = mybir.dt.bfloat16
AFT = mybir.ActivationFunctionType

_NC_CACHE = None


def build_nc():
    nc = bacc.Bacc("TRN2", target_bir_lowering=False, debug=False,
                   num_devices=N_CORES)

    MCH_ = 3 * C // P
    # x pre-cast to bf16 host-side (the kernel computes in bf16 anyway);
    # halves the DMA bytes and removes the on-device cast
    x = nc.declare_dram_parameter("x_bf", [T, C], bf16, isOutput=False)
    # w_attn host-repacked (bf16) so chunk m is one contiguous 4KB run per
    # partition: w_pk[p, m, kk, j] = w_attn[kk*128+p, m*128+j]
    w_attn = nc.declare_dram_parameter("w_attn_pk", [P, MCH_, KK, P], bf16,
                                       isOutput=False)
    # b_attn pre-arranged host-side to [P, 48] (partition-major chunks,
    # q-columns pre-scaled by 1/sqrt(HD))
    b_attn = nc.declare_dram_parameter("b_attn_pm", [P, 3 * C // P], f32,
                                       isOutput=False)
    # w_proj host-repacked (bf16): w_proj_pk[p, ct, kk, j] = w_proj[kk*128+p, ct*512+j]
    w_proj = nc.declare_dram_parameter("w_proj_pk", [P, C // TQ, KK, TQ], bf16,
                                       isOutput=False)
    # b_proj broadcast to all 128 partitions host-side
    bias_bc = nc.declare_dram_parameter("bias_bc", [P, C], f32, isOutput=False)
    masks = nc.declare_dram_parameter("masks", [P, P], bf16, isOutput=False)
    ident_b = nc.declare_dram_parameter("ident_b", [P, P], bf16, isOutput=False)
    ones_b = nc.declare_dram_parameter("ones_b", [P, P], bf16, isOutput=False)
    y = nc.declare_dram_parameter("y", [T, C], f32, isOutput=True)

    MCH = 3 * C // P  # 48 output chunks of qkv^T

    # m-chunk processing order: v first (PV of head 0 needs all of v), then
    # (k_h, q_h) pairs so head h's S-matmuls unblock as early as possible.
    m_order = list(range(32, 48))
    for h in range(H):
        m_order.append(16 + h)
        m_order.append(h)
    FIRSTG = m_order[:4]  # chunks whose qt=0 sweep runs early, during x DMA

    with tile.TileContext(nc) as tc:
        with tc.tile_pool(name="consts", bufs=1) as consts, \
             tc.tile_pool(name="resid", bufs=1) as resid:

            # ---- constants: only the transpose identity is needed right
            # away (scalar queue, ahead of x0); everything else is deferred
            # so the x tiles go first on their queues ----
            identb_sb = consts.tile([P, P], bf16, tag="identb", name="identb")
            nc.scalar.dma_start(out=identb_sb, in_=ident_b[:])
            ones_sb = consts.tile([P, P], bf16, tag="ones", name="ones")
            batt_sb = consts.tile([P, MCH], f32, tag="batt", name="batt")
            masks_sb = consts.tile([P, P], bf16, tag="masks", name="masks")
            bias_sb = consts.tile([P, C], f32, tag="biasbc", name="biasbc")

            # ---- persistent intermediates (bf16) ----
            qT = [resid.tile([P, T], bf16, tag=f"qT{i}", name=f"qT{i}") for i in range(H)]
            kT = [resid.tile([P, T], bf16, tag=f"kT{i}", name=f"kT{i}") for i in range(H)]
            # v in natural layout, one tile: vv[:, kt, :] = v rows kt*128..
            vv = resid.tile([P, TT, C], bf16, tag="vv", name="vv")

            with tc.tile_pool(name="xT", bufs=1) as xTp, \
                 tc.tile_pool(name="wst", bufs=4) as wst, \
                 tc.tile_pool(name="vtp", bufs=2) as vtp, \
                 tc.tile_pool(name="psA", bufs=2, space=bass.MemorySpace.PSUM) as psA, \
                 tc.tile_pool(name="psB", bufs=3, space=bass.MemorySpace.PSUM) as psB:

                # xT split into q-halves: xTh[0][:, c, :] covers x rows
                # 0..511, xTh[1] rows 512..1023 (so qt=0 work unblocks early)
                xTh = [xTp.tile([P, KK, TQ], bf16, tag=f"xTh{q}",
                                name=f"xTh{q}") for q in range(NQ)]

                def w_load(m):
                    wbf = wst.tile([P, KK, P], bf16, tag="wbf", name="wbf")
                    nc.sync.dma_start(out=wbf, in_=w_attn[:, m, :, :])
                    return wbf

                def vtile(m):
                    return vtp.tile([P, T], bf16, tag="vtmp",
                                    name="vtmp") if m >= 32 else None

                def qkv_sweep(m, qts, wbf, vdest):
                    """QKV sweeps for the given qt halves of chunk m, packed
                    into one PSUM tile with a single fused bias/scale/cast
                    activation covering all of them."""
                    ps = psB.tile([P, NQ * TQ], f32, tag="psB", name="psB")
                    for qt in qts:
                        for kk in range(KK):
                            nc.tensor.matmul(
                                ps[:, qt * TQ:(qt + 1) * TQ],
                                wbf[:, kk, :], xTh[qt][:, kk, :],
                                start=(kk == 0), stop=(kk == KK - 1))
                    sc = SCALE if m < 16 else 1.0
                    bias_ap = batt_sb[:, m:m + 1]
                    lo, hi = qts[0] * TQ, (qts[-1] + 1) * TQ
                    if m < 16:
                        dest = qT[m]
                    elif m < 32:
                        dest = kT[m - 16]
                    else:
                        dest = vdest
                    nc.scalar.activation(out=dest[:, lo:hi], in_=ps[:, lo:hi],
                                         func=AFT.Identity, bias=bias_ap,
                                         scale=sc)
                    if m >= 32:
                        # transpose the 4 key-blocks of each half into one
                        # PSUM bank, then one strided DVE copy into vv
                        h = m - 32
                        for qt in qts:
                            pv = psA.tile([P, 8 * P], bf16, tag="pst",
                                          name="pst")
                            for k4 in range(TQ // P):
                                nc.tensor.matmul(
                                    pv[:, k4 * P:(k4 + 1) * P],
                                    vdest[:, qt * TQ + k4 * P:
                                          qt * TQ + (k4 + 1) * P], identb_sb,
                                    is_transpose=True,
                                    start=(k4 == 0), stop=(k4 == TQ // P - 1))
                            nc.vector.tensor_copy(
                                vv[:, qt * 4:(qt + 1) * 4, h * P:(h + 1) * P],
                                pv[:, 0:TQ])

                # ---- Phase A: load x bf16 (split across the scalar and
                # gpsimd DMA queues, parallel to the weight loads on the sync
                # queue), PE-transpose; interleaved with the first-group qt=0
                # sweeps so the PE has work while the tail of x streams in ----
                with tc.tile_pool(name="ldx", bufs=4) as ldx:

                    x_tiles = {}

                    def x_dma(t):
                        x_sb = ldx.tile([P, C], bf16, tag="x_sb", name="x_sb")
                        eng = nc.scalar if t % 2 == 0 else nc.gpsimd
                        eng.dma_start(out=x_sb, in_=x[t * P:(t + 1) * P, :])
                        x_tiles[t] = x_sb

                    def transpose_tile(t):
                        if t + 4 < TT:
                            x_dma(t + 4)
                        x_bf = x_tiles.pop(t)
                        qh, tq = divmod(t, TT // NQ)
                        for g in range(2):  # two banks of 8 transposes each
                            pt = psA.tile([P, 8 * P], bf16, tag="pst",
                                          name="pst")
                            for j in range(8):
                                c = g * 8 + j
                                nc.tensor.matmul(
                                    pt[:, j * P:(j + 1) * P],
                                    x_bf[:, c * P:(c + 1) * P], identb_sb,
                                    is_transpose=True,
                                    start=(j == 0), stop=(j == 7))
                            nc.vector.tensor_copy(
                                xTh[qh][:, g * 8:(g + 1) * 8,
                                        tq * P:(tq + 1) * P], pt)

                    for t in range(4):
                        x_dma(t)
                    # mid-priority constants, after the first x tiles
                    nc.gpsimd.dma_start(out=batt_sb, in_=b_attn[:])
                    nc.gpsimd.dma_start(out=ones_sb, in_=ones_b[:])
                    wbf_first = {}
                    for t in range(4):
                        transpose_tile(t)
                        if t < len(FIRSTG):
                            # start the W loads early (parallel queue)
                            wbf_first[FIRSTG[t]] = w_load(FIRSTG[t])
                    for i, m in enumerate(FIRSTG):
                        qkv_sweep(m, [0], wbf_first[m], vtile(m))
                        if 4 + i < TT:
                            transpose_tile(4 + i)

                # deferred constant loads (gpsimd queue is now clear of x)
                nc.gpsimd.dma_start(out=masks_sb, in_=masks[:])
                nc.gpsimd.dma_start(out=bias_sb, in_=bias_bc[:])

                # ---- Phase B: remaining qkv^T work (first-group chunks are
                # re-fetched; only their qt=1 sweep remains) ----
                for m in m_order:
                    wbf = w_load(m)
                    if m in FIRSTG:
                        qkv_sweep(m, [1], wbf, vtile(m))
                    else:
                        qkv_sweep(m, [0, 1], wbf, vtile(m))

            # ---- Phase C: attention per head ----
            with tc.tile_pool(name="attnp", bufs=1) as attnp, \
                 tc.tile_pool(name="wpc", bufs=2) as wpc:
                attnT = [attnp.tile([P, T], bf16, tag=f"attnT{i}", name=f"attnT{i}")
                         for i in range(H)]

                def d_load(ct):
                    """DMA the full [C, 512] bf16 w_proj slice for ct."""
                    wpbf = wpc.tile([P, KK, TQ], bf16, tag="wpbf",
                                    name="wpbf")
                    nc.sync.dma_start(out=wpbf, in_=w_proj[:, ct, :, :])
                    return wpbf

                # prefetch the first w_proj slice during phase C (its SBUF
                # zone comes from phase-B frees, so the DMA runs mid-C)
                wp0 = d_load(0)

                with tc.tile_pool(name="eSp", bufs=2) as eSp, \
                     tc.tile_pool(name="ePp", bufs=3) as ePp, \
                     tc.tile_pool(name="ctmp", bufs=2) as ctmp, \
                     tc.tile_pool(name="psS", bufs=2, space=bass.MemorySpace.PSUM) as psS, \
                     tc.tile_pool(name="psO", bufs=1, space=bass.MemorySpace.PSUM) as psO, \
                     tc.tile_pool(name="psD", bufs=2, space=bass.MemorySpace.PSUM) as psD:
                    NPAIR = TT // 2  # 4 kt-pairs per head
                    # packed causal eS: row kt holds q in [kt*128, 1024)
                    # (width 1024-128*kt), rows concatenated tightly
                    WROW = [T - P * kt for kt in range(TT)]
                    SROW = [sum(WROW[:kt]) for kt in range(TT)]
                    ES_W = sum(WROW)  # 4608

                    def pk(eS, kt, q0, q1):
                        base = SROW[kt] - P * kt
                        return eS[:, base + q0:base + q1]

                    # packed eP: pair i holds q in [2i*128, 1024)
                    WPAIR = [T - 2 * P * i for i in range(NPAIR)]
                    SPAIR = [sum(WPAIR[:i]) for i in range(NPAIR)]
                    EP_W = sum(WPAIR)  # 2560

                    def ep(eP, i, q0, q1):
                        base = SPAIR[i] - 2 * P * i
                        return eP[:, base + q0:base + q1]

                    def c_front(h, pv_h, pv_eS):
                        """Scores/exp/masks/pair-sums for head h, with the PV
                        matmuls of head pv_h interleaved between score groups
                        so the PE always has ready work while ACT catches up.
                        All score matmuls and exps are narrowed to the
                        causally valid columns q >= kt*128."""
                        eS = eSp.tile([P, ES_W], bf16, tag="eS", name="eS")
                        pso = None
                        if pv_h is not None:
                            pso = psO.tile([P, NQ, TQ], f32, tag="psO",
                                           name="psO")

                        def pv_mms(kts):
                            if pv_h is None:
                                return
                            for kt in kts:
                                for qt in range(NQ):
                                    q0 = max(qt * TQ, kt * P)
                                    q1 = (qt + 1) * TQ
                                    if q0 >= q1:
                                        continue
                                    nc.tensor.matmul(
                                        pso[:, qt, q0 - qt * TQ:TQ],
                                        vv[:, kt, pv_h * P:(pv_h + 1) * P],
                                        pk(pv_eS, kt, q0, q1),
                                        start=(kt == 0),
                                        stop=(kt == min(7, qt * 4 + 3)))

                        eP = ePp.tile([P, EP_W], bf16, tag="eP", name="eP")

                        def mask(kt):
                            # the causal triangle is the first 128 packed
                            # columns of every row; alternate engines
                            esl = eS[:, SROW[kt]:SROW[kt] + P]
                            eng = nc.vector if kt % 2 == 0 else nc.gpsimd
                            eng.tensor_mul(esl, esl, masks_sb)

                        def pair_add(i):
                            # eP[i] = eS[2i] + eS[2i+1] on the union of their
                            # valid ranges: copy the 128-wide prefix where
                            # only 2i is valid, add the rest
                            ka, kb = 2 * i, 2 * i + 1
                            qa, qb = ka * P, kb * P
                            eng = nc.gpsimd if i % 2 == 0 else nc.vector
                            eng.tensor_copy(ep(eP, i, qa, qb),
                                            pk(eS, ka, qa, qb))
                            eng.tensor_add(ep(eP, i, qb, T),
                                           pk(eS, ka, qb, T),
                                           pk(eS, kb, qb, T))

                        for kt in range(4):  # one psum/exp group per kt
                            pss = psS.tile([P, 2 * TQ], f32, tag="psS",
                                           name="psS")
                            nc.tensor.matmul(
                                pss[:, kt * P:TQ],
                                kT[h][:, kt * P:(kt + 1) * P],
                                qT[h][:, kt * P:TQ],
                                start=True, stop=True)
                            nc.tensor.matmul(
                                pss[:, TQ:2 * TQ],
                                kT[h][:, kt * P:(kt + 1) * P],
                                qT[h][:, TQ:2 * TQ],
                                start=True, stop=True)
                            nc.scalar.activation(
                                out=pk(eS, kt, kt * P, T),
                                in_=pss[:, kt * P:2 * TQ], func=AFT.Exp)
                            mask(kt)
                            if kt % 2 == 1:
                                pair_add(kt // 2)
                            pv_mms([kt])
                        # g4: kt4 (full) + kt5 (384 cols) in one 2-bank psum
                        pss = psS.tile([P, 2 * TQ], f32, tag="psS", name="psS")
                        nc.tensor.matmul(pss[:, 0:TQ],
                                         kT[h][:, 4 * P:5 * P],
                                         qT[h][:, TQ:2 * TQ],
                                         start=True, stop=True)
                        nc.tensor.matmul(pss[:, TQ:TQ + 384],
                                         kT[h][:, 5 * P:6 * P],
                                         qT[h][:, TQ + P:2 * TQ],
                                         start=True, stop=True)
                        nc.scalar.activation(
                            out=eS[:, SROW[4]:SROW[4] + 896],
                            in_=pss[:, 0:896], func=AFT.Exp)
                        mask(4)
                        mask(5)
                        pair_add(2)
                        pv_mms([4, 5])
                        # g5: kt6 (256) + kt7 (128) packed into one bank
                        pss = psS.tile([P, 2 * TQ], f32, tag="psS", name="psS")
                        nc.tensor.matmul(pss[:, 0:256],
                                         kT[h][:, 6 * P:7 * P],
                                         qT[h][:, TQ + 2 * P:2 * TQ],
                                         start=True, stop=False)
                        nc.tensor.matmul(pss[:, 256:384],
                                         kT[h][:, 7 * P:8 * P],
                                         qT[h][:, TQ + 3 * P:2 * TQ],
                                         start=False, stop=True)
                        nc.scalar.activation(
                            out=eS[:, SROW[6]:ES_W],
                            in_=pss[:, 0:384], func=AFT.Exp)
                        mask(6)
                        mask(7)
                        pair_add(3)
                        pv_mms([6, 7])
                        return eS, eP, pso

                    def c_denom_div(h, eP, pso):
                        """Denominators from pair-sums (narrowed to each
                        pair's valid queries), then divide."""
                        psd = [psD.tile([P, TQ], f32, tag="psD", name="psD")
                               for _ in range(NQ)]
                        # pair i is valid for q >= 2i*128
                        for qt in range(NQ):
                            lastp = 1 if qt == 0 else NPAIR - 1
                            for i in range(lastp + 1):
                                q0 = max(qt * TQ, 2 * i * P)
                                q1 = (qt + 1) * TQ
                                nc.tensor.matmul(
                                    psd[qt][:, q0 - qt * TQ:TQ], ones_sb,
                                    ep(eP, i, q0, q1),
                                    start=(i == 0), stop=(i == lastp))
                        for qt in range(NQ):
                            rec = ctmp.tile([P, TQ], f32, tag="rec", name="rec")
                            # ~18-bit accurate, 5x faster than reciprocal();
                            # denominators are in [1, ~2e5] so edge cases are
                            # impossible
                            nc.vector.reciprocal_approx_fast(out=rec,
                                                             in_=psd[qt])
                            nc.vector.tensor_mul(
                                attnT[h][:, qt * TQ:(qt + 1) * TQ],
                                pso[:, qt, :], rec)

                    # Software-pipelined: PV runs one head behind the scores
                    # (interleaved between score groups), denominators two
                    # heads behind. The denom/div of head h-2 is emitted first
                    # so its divide frees the single PV PSUM slot before the
                    # PV matmuls of head h-1 need it.
                    st = {}
                    for h in range(H):
                        if h >= 2:
                            c_denom_div(h - 2, st[h - 2][1], st[h - 1][2])
                        prev_eS = st[h - 1][0] if h >= 1 else None
                        st[h] = c_front(h, h - 1 if h >= 1 else None, prev_eS)
                    c_denom_div(H - 2, st[H - 2][1], st[H - 1][2])
                    # final head's PV + denom/div
                    pso_last = psO.tile([P, NQ, TQ], f32, tag="psO", name="psO")
                    eS_last = st[H - 1][0]
                    for kt in range(TT):
                        for qt in range(NQ):
                            q0 = max(qt * TQ, kt * P)
                            q1 = (qt + 1) * TQ
                            if q0 >= q1:
                                continue
                            nc.tensor.matmul(
                                pso_last[:, qt, q0 - qt * TQ:TQ],
                                vv[:, kt, (H - 1) * P:H * P],
                                pk(eS_last, kt, q0, q1),
                                start=(kt == 0), stop=(kt == min(7, qt * 4 + 3)))
                    c_denom_div(H - 1, st[H - 1][1], pso_last)

                # ---- Phase D: output projection (ct-outer, t-half groups) ----
                with tc.tile_pool(name="ybuf", bufs=4) as ybuf, \
                     tc.tile_pool(name="psY", bufs=8, space=bass.MemorySpace.PSUM) as psYp:
                    NCT = C // TQ  # 4

                    wp = wp0
                    for ct in range(NCT):
                        wp_next = d_load(ct + 1) if ct + 1 < NCT else None
                        for half in range(2):
                            ts = range(half * 4, half * 4 + 4)
                            psY = {t: psYp.tile([P, TQ], f32, tag="psY",
                                                name="psY") for t in ts}
                            for kk in range(KK):
                                for t in ts:
                                    nc.tensor.matmul(
                                        psY[t], attnT[kk][:, t * P:(t + 1) * P],
                                        wp[:, kk, :],
                                        start=(kk == 0), stop=(kk == KK - 1))
                            for t in ts:
                                y_sb = ybuf.tile([P, TQ], f32, tag="y_sb",
                                                 name="y_sb")
                                nc.vector.tensor_add(
                                    y_sb, psY[t],
                                    bias_sb[:, ct * TQ:(ct + 1) * TQ])
                                # y stores alternate between the gpsimd and
                                # scalar DMA queues (parallel to the w_proj
                                # loads on the sync queue)
                                eng = nc.gpsimd if t % 2 == 0 else nc.scalar
                                eng.dma_start(
                                    out=y[t * P:(t + 1) * P,
                                          ct * TQ:(ct + 1) * TQ],
                                    in_=y_sb)
                        wp = wp_next

    nc.compile()
    return nc


def _get_nc():
    global _NC_CACHE
    if _NC_CACHE is None:
        _NC_CACHE = build_nc()
    return _NC_CACHE


def make_in_maps(inputs):
    x = np.asarray(inputs["x"], dtype=np.float32)
    w_attn = np.asarray(inputs["w_attn"], dtype=np.float32)
    b_attn = np.asarray(inputs["b_attn"], dtype=np.float32)
    w_proj = np.asarray(inputs["w_proj"], dtype=np.float32)
    b_proj = np.asarray(inputs["b_proj"], dtype=np.float32)

    # cast to bf16 host-side (device computes in bf16 anyway) and repack
    # weights so each SBUF partition reads one contiguous block
    MCH = 3 * C // P
    x_bf = np.ascontiguousarray(x.astype(ml_dtypes.bfloat16))
    w_attn_pk = np.ascontiguousarray(
        w_attn.astype(ml_dtypes.bfloat16)
        .reshape(KK, P, MCH, P).transpose(1, 2, 0, 3))
    w_proj_pk = np.ascontiguousarray(
        w_proj.astype(ml_dtypes.bfloat16)
        .reshape(KK, P, C // TQ, TQ).transpose(1, 2, 0, 3))

    # bias prep: [3C] -> [P, 48] partition-major; q columns folded with scale
    bpm = np.ascontiguousarray(b_attn.reshape(3 * C // P, P).T).copy()
    bpm[:, :16] *= SCALE
    bias_bc = np.ascontiguousarray(
        np.broadcast_to(b_proj.reshape(1, C), (P, C)).astype(np.float32))

    # single [128,128] causal triangle (valid where q_local >= k_local)
    masks = (np.arange(P)[None, :] >= np.arange(P)[:, None]).astype(
        ml_dtypes.bfloat16)
    ident_b = np.eye(P, dtype=ml_dtypes.bfloat16)
    ones_b = np.ones((P, P), dtype=ml_dtypes.bfloat16)

    common = dict(w_attn_pk=w_attn_pk, b_attn_pm=bpm, w_proj_pk=w_proj_pk,
                  bias_bc=bias_bc, masks=masks,
                  ident_b=ident_b, ones_b=ones_b)
    return [dict(x_bf=np.ascontiguousarray(x_bf[i]), **common)
            for i in range(B)]


def run_spmd(inputs, trace=False, **kw):
    nc = _get_nc()
    in_maps = make_in_maps(inputs)
    return run_bass_kernel_spmd(nc, in_maps, list(range(N_CORES)),
                                trace=trace, **kw)


def kernel(**inputs):
    res = run_spmd(inputs, trace=False)
    y = np.stack([np.asarray(res.results[i]["y"]) for i in range(N_CORES)])
    return y.astype(np.float32)


if __name__ == "__main__":
    rng = np.random.default_rng(0)
    demo = {
        "x": rng.standard_normal((B, T, C)).astype(np.float32),
        "w_attn": (rng.standard_normal((C, 3 * C)) * 0.02).astype(np.float32),
        "b_attn": (rng.standard_normal(3 * C) * 0.02).astype(np.float32),
        "w_proj": (rng.standard_normal((C, C)) * 0.02).astype(np.float32),
        "b_proj": (rng.standard_normal(C) * 0.02).astype(np.float32),
    }
    out = kernel(**demo)
    print("out", out.shape, out.dtype, float(np.abs(out).max()))


# revision 53
# speedup vs baseline: 1.0185x; 1.0185x over previous
"""Causal self-attention (B=8, T=1024, C=2048, H=16) on 8 TRN2 NeuronCores.

Strategy: data-parallel over batch — core i computes the full attention block
for batch element i (weights replicated, no collectives).

Per-core pipeline (Tile framework, all matmuls bf16 on the PE):
  A) x [T,C] f32 -> ACT cast to bf16 -> PE-transpose (bf16, 1cyc/row) into
     xT halves (xTa = rows 0..511, xTb = rows 512..1023) so phase B's qt=0
     matmuls can start once half of x has landed.
  B) qkv^T = W-chunk-stationary matmuls vs xT moving; PSUM->SBUF copies fuse
     bias (+ softmax scale for q) and cast to bf16. The first 4 v-chunks run
     their qt=0 sweep early (interleaved with the tail of phase A) to keep the
     PE busy during the x DMA. v-chunks are produced transposed and
     PE-transposed back to natural [T, C] layout.
  C) per head: S^T blocks (kT-chunk stationary, qT moving), pair-packed into
     [128,1024] PSUM tiles so one ACT exp covers two blocks; causal masking of
     diagonal blocks via precomputed 0/1 masks on DVE; eS block PAIRS summed on
     the (otherwise idle) GPSIMD engine so the ones-matmul denominators need
     half the PE streaming; PV accumulates out^T; divide by denom on DVE.
  D) y = attnT-stationary @ w_proj, ct-outer with the full [C,512] w_proj
     slice resident in SBUF (bf16) and two 4-bank PSUM t-half groups rotating
     so ct boundaries overlap; bias added on DVE from a host-precomputed
     broadcast tile during the PSUM->SBUF copy; output f32.
"""

import sys

if "/opt/trn_rl_repo" not in sys.path:
    sys.path.insert(0, "/opt/trn_rl_repo")

import numpy as np
import ml_dtypes

import concourse.bass as bass
import concourse.mybir as mybir
import concourse.tile as tile
from concourse import bacc
from concourse.bass_utils import run_bass_kernel_spmd

B, T, C = 8, 1024, 2048
H, HD = 16, 128
N_CORES = 8
P = 128            # partition dim
TQ = 512           # moving-operand tile (q positions per matmul)
KK = C // P        # 16 contraction tiles over C
TT = T // P        # 8 tiles over T
NQ = T // TQ       # 2 q-tiles
SCALE = 1.0 / float(np.sqrt(HD))

f32 = mybir.dt.float32
bf16 = mybir.dt.bfloat16
AFT = mybir.ActivationFunctionType

_NC_CACHE = None


def build_nc():
    nc = bacc.Bacc("TRN2", target_bir_lowering=False, debug=False,
                   num_devices=N_CORES)

    MCH_ = 3 * C // P
    # x pre-cast to bf16 host-side (the kernel computes in bf16 anyway);
    # halves the DMA bytes and removes the on-device cast
    x = nc.declare_dram_parameter("x_bf", [T, C], bf16, isOutput=False)
    # w_attn host-repacked (bf16) so chunk m is one contiguous 4KB run per
    # partition: w_pk[p, m, kk, j] = w_attn[kk*128+p, m*128+j]
    w_attn = nc.declare_dram_parameter("w_attn_pk", [P, MCH_, KK, P], bf16,
                                       isOutput=False)
    # b_attn pre-arranged host-side to [P, 48] (partition-major chunks,
    # q-columns pre-scaled by 1/sqrt(HD))
    b_attn = nc.declare_dram_parameter("b_attn_pm", [P, 3 * C // P], f32,
                                       isOutput=False)
    # w_proj host-repacked (bf16): w_proj_pk[p, ct, kk, j] = w_proj[kk*128+p, ct*512+j]
    w_proj = nc.declare_dram_parameter("w_proj_pk", [P, C // TQ, KK, TQ], bf16,
                                       isOutput=False)
    # b_proj broadcast to all 128 partitions host-side
    bias_bc = nc.declare_dram_parameter("bias_bc", [P, C], f32, isOutput=False)
    masks = nc.declare_dram_parameter("masks", [P, P], bf16, isOutput=False)
    ident_b = nc.declare_dram_parameter("ident_b", [P, P], bf16, isOutput=False)
    ones_b = nc.declare_dram_parameter("ones_b", [P, P], bf16, isOutput=False)
    y = nc.declare_dram_parameter("y", [T, C], f32, isOutput=True)

    MCH = 3 * C // P  # 48 output chunks of qkv^T

    # m-chunk processing order: v first (PV of head 0 needs all of v), then
    # (k_h, q_h) pairs so head h's S-matmuls unblock as early as possible.
    m_order = list(range(32, 48))
    for h in range(H):
        m_order.append(16 + h)
        m_order.append(h)
    FIRSTG = m_order[:4]  # chunks whose qt=0 sweep runs early, during x DMA

    with tile.TileContext(nc) as tc:
        with tc.tile_pool(name="consts", bufs=1) as consts, \
             tc.tile_pool(name="resid", bufs=1) as resid:

            # ---- constants: only the transpose identity is needed right
            # away (scalar queue, ahead of x0); everything else is deferred
            # so the x tiles go first on their queues ----
            identb_sb = consts.tile([P, P], bf16, tag="identb", name="identb")
            nc.scalar.dma_start(out=identb_sb, in_=ident_b[:])
            ones_sb = consts.tile([P, P], bf16, tag="ones", name="ones")
            batt_sb = consts.tile([P, MCH], f32, tag="batt", name="batt")
            masks_sb = consts.tile([P, P], bf16, tag="masks", name="masks")
            bias_sb = consts.tile([P, C], f32, tag="biasbc", name="biasbc")

            # ---- persistent intermediates (bf16) ----
            qT = [resid.tile([P, T], bf16, tag=f"qT{i}", name=f"qT{i}") for i in range(H)]
            kT = [resid.tile([P, T], bf16, tag=f"kT{i}", name=f"kT{i}") for i in range(H)]
            # v in natural layout, one tile: vv[:, kt, :] = v rows kt*128..
            vv = resid.tile([P, TT, C], bf16, tag="vv", name="vv")

            with tc.tile_pool(name="xT", bufs=1) as xTp, \
                 tc.tile_pool(name="wst", bufs=4) as wst, \
                 tc.tile_pool(name="vtp", bufs=2) as vtp, \
                 tc.tile_pool(name="psA", bufs=2, space=bass.MemorySpace.PSUM) as psA, \
                 tc.tile_pool(name="psB", bufs=3, space=bass.MemorySpace.PSUM) as psB:

                # xT split into q-halves: xTh[0][:, c, :] covers x rows
                # 0..511, xTh[1] rows 512..1023 (so qt=0 work unblocks early)
                xTh = [xTp.tile([P, KK, TQ], bf16, tag=f"xTh{q}",
                                name=f"xTh{q}") for q in range(NQ)]

                def w_load(m):
                    wbf = wst.tile([P, KK, P], bf16, tag="wbf", name="wbf")
                    nc.sync.dma_start(out=wbf, in_=w_attn[:, m, :, :])
                    return wbf

                def vtile(m):
                    return vtp.tile([P, T], bf16, tag="vtmp",
                                    name="vtmp") if m >= 32 else None

                def qkv_sweep(m, qts, wbf, vdest):
                    """QKV sweeps for the given qt halves of chunk m, packed
                    into one PSUM tile with a single fused bias/scale/cast
                    activation covering all of them."""
                    ps = psB.tile([P, NQ * TQ], f32, tag="psB", name="psB")
                    for qt in qts:
                        for kk in range(KK):
                            nc.tensor.matmul(
                                ps[:, qt * TQ:(qt + 1) * TQ],
                                wbf[:, kk, :], xTh[qt][:, kk, :],
                                start=(kk == 0), stop=(kk == KK - 1))
                    sc = SCALE if m < 16 else 1.0
                    bias_ap = batt_sb[:, m:m + 1]
                    lo, hi = qts[0] * TQ, (qts[-1] + 1) * TQ
                    if m < 16:
                        dest = qT[m]
                    elif m < 32:
                        dest = kT[m - 16]
                    else:
                        dest = vdest
                    nc.scalar.activation(out=dest[:, lo:hi], in_=ps[:, lo:hi],
                                         func=AFT.Identity, bias=bias_ap,
                                         scale=sc)
                    if m >= 32:
                        # transpose the 4 key-blocks of each half into one
                        # PSUM bank, then one strided DVE copy into vv
                        h = m - 32
                        for qt in qts:
                            pv = psA.tile([P, 8 * P], bf16, tag="pst",
                                          name="pst")
                            for k4 in range(TQ // P):
                                nc.tensor.matmul(
                                    pv[:, k4 * P:(k4 + 1) * P],
                                    vdest[:, qt * TQ + k4 * P:
                                          qt * TQ + (k4 + 1) * P], identb_sb,
                                    is_transpose=True,
                                    start=(k4 == 0), stop=(k4 == TQ // P - 1))
                            nc.vector.tensor_copy(
                                vv[:, qt * 4:(qt + 1) * 4, h * P:(h + 1) * P],
                                pv[:, 0:TQ])

                # ---- Phase A: load x bf16 (split across the scalar and
                # gpsimd DMA queues, parallel to the weight loads on the sync
                # queue), PE-transpose; interleaved with the first-group qt=0
                # sweeps so the PE has work while the tail of x streams in ----
                with tc.tile_pool(name="ldx", bufs=4) as ldx:

                    x_tiles = {}

                    def x_dma(t):
                        x_sb = ldx.tile([P, C], bf16, tag="x_sb", name="x_sb")
                        eng = nc.scalar if t % 2 == 0 else nc.gpsimd
                        eng.dma_start(out=x_sb, in_=x[t * P:(t + 1) * P, :])
                        x_tiles[t] = x_sb

                    def transpose_tile(t):
                        if t + 4 < TT:
                            x_dma(t + 4)
                        x_bf = x_tiles.pop(t)
                        qh, tq = divmod(t, TT // NQ)
                        for g in range(2):  # two banks of 8 transposes each
                            pt = psA.tile([P, 8 * P], bf16, tag="pst",
                                          name="pst")
                            for j in range(8):
                                c = g * 8 + j
                                nc.tensor.matmul(
                                    pt[:, j * P:(j + 1) * P],
                                    x_bf[:, c * P:(c + 1) * P], identb_sb,
                                    is_transpose=True,
                                    start=(j == 0), stop=(j == 7))
                            nc.vector.tensor_copy(
                                xTh[qh][:, g * 8:(g + 1) * 8,
                                        tq * P:(tq + 1) * P], pt)

                    for t in range(4):
                        x_dma(t)
                    # mid-priority constants, after the first x tiles
                    nc.gpsimd.dma_start(out=batt_sb, in_=b_attn[:])
                    nc.gpsimd.dma_start(out=ones_sb, in_=ones_b[:])
                    wbf_first = {}
                    for t in range(4):
                        transpose_tile(t)
                        if t < len(FIRSTG):
                            # start the W loads early (parallel queue)
                            wbf_first[FIRSTG[t]] = w_load(FIRSTG[t])
                    for i, m in enumerate(FIRSTG):
                        qkv_sweep(m, [0], wbf_first[m], vtile(m))
                        if 4 + i < TT:
                            transpose_tile(4 + i)

                # deferred constant loads (gpsimd queue is now clear of x)
                nc.gpsimd.dma_start(out=masks_sb, in_=masks[:])
                nc.gpsimd.dma_start(out=bias_sb, in_=bias_bc[:])

                # ---- Phase B: remaining qkv^T work (first-group chunks are
                # re-fetched; only their qt=1 sweep remains) ----
                for m in m_order:
                    wbf = w_load(m)
                    if m in FIRSTG:
                        qkv_sweep(m, [1], wbf, vtile(m))
                    else:
                        qkv_sweep(m, [0, 1], wbf, vtile(m))

            # ---- Phase C: attention per head ----
            with tc.tile_pool(name="attnp", bufs=1) as attnp, \
                 tc.tile_pool(name="wpc", bufs=2) as wpc:
                attnT = [attnp.tile([P, T], bf16, tag=f"attnT{i}", name=f"attnT{i}")
                         for i in range(H)]

                def d_load(ct):
                    """DMA the full [C, 512] bf16 w_proj slice for ct."""
                    wpbf = wpc.tile([P, KK, TQ], bf16, tag="wpbf",
                                    name="wpbf")
                    nc.sync.dma_start(out=wpbf, in_=w_proj[:, ct, :, :])
                    return wpbf

                # prefetch the first w_proj slice during phase C (its SBUF
                # zone comes from phase-B frees, so the DMA runs mid-C)
                wp0 = d_load(0)

                with tc.tile_pool(name="eSp", bufs=2) as eSp, \
                     tc.tile_pool(name="ePp", bufs=3) as ePp, \
                     tc.tile_pool(name="ctmp", bufs=2) as ctmp, \
                     tc.tile_pool(name="psS", bufs=2, space=bass.MemorySpace.PSUM) as psS, \
                     tc.tile_pool(name="psO", bufs=1, space=bass.MemorySpace.PSUM) as psO, \
                     tc.tile_pool(name="psD", bufs=2, space=bass.MemorySpace.PSUM) as psD:
                    NPAIR = TT // 2  # 4 kt-pairs per head
                    # packed causal eS: row kt holds q in [kt*128, 1024)
                    # (width 1024-128*kt), rows concatenated tightly
                    WROW = [T - P * kt for kt in range(TT)]
                    SROW = [sum(WROW[:kt]) for kt in range(TT)]
                    ES_W = sum(WROW)  # 4608

                    def pk(eS, kt, q0, q1):
                        base = SROW[kt] - P * kt
                        return eS[:, base + q0:base + q1]

                    # packed eP: pair i holds q in [2i*128, 1024)
                    WPAIR = [T - 2 * P * i for i in range(NPAIR)]
                    SPAIR = [sum(WPAIR[:i]) for i in range(NPAIR)]
                    EP_W = sum(WPAIR)  # 2560

                    def ep(eP, i, q0, q1):
                        base = SPAIR[i] - 2 * P * i
                        return eP[:, base + q0:base + q1]

                    def c_front(h, pv_h, pv_eS):
                        """Scores/exp/masks/pair-sums for head h, with the PV
                        matmuls of head pv_h interleaved between score groups
                        so the PE always has ready work while ACT catches up.
                        All score matmuls and exps are narrowed to the
                        causally valid columns q >= kt*128."""
                        eS = eSp.tile([P, ES_W], bf16, tag="eS", name="eS")
                        pso = None
                        if pv_h is not None:
                            pso = psO.tile([P, NQ, TQ], f32, tag="psO",
                                           name="psO")

                        def pv_mms(kts):
                            if pv_h is None:
                                return
                            for kt in kts:
                                for qt in range(NQ):
                                    q0 = max(qt * TQ, kt * P)
                                    q1 = (qt + 1) * TQ
                                    if q0 >= q1:
                                        continue
                                    nc.tensor.matmul(
                                        pso[:, qt, q0 - qt * TQ:TQ],
                                        vv[:, kt, pv_h * P:(pv_h + 1) * P],
                                        pk(pv_eS, kt, q0, q1),
                                        start=(kt == 0),
                                        stop=(kt == min(7, qt * 4 + 3)))

                        eP = ePp.tile([P, EP_W], bf16, tag="eP", name="eP")

                        def mask(kt):
                            # the causal triangle is the first 128 packed
                            # columns of every row
                            esl = eS[:, SROW[kt]:SROW[kt] + P]
                            nc.vector.tensor_mul(esl, esl, masks_sb)

                        def pair_add(i):
                            # eP[i] = eS[2i] + eS[2i+1] on the union of their
                            # valid ranges: copy the 128-wide prefix where
                            # only 2i is valid, add the rest
                            ka, kb = 2 * i, 2 * i + 1
                            qa, qb = ka * P, kb * P
                            eng = nc.gpsimd if i % 2 == 0 else nc.vector
                            eng.tensor_copy(ep(eP, i, qa, qb),
                                            pk(eS, ka, qa, qb))
                            eng.tensor_add(ep(eP, i, qb, T),
                                           pk(eS, ka, qb, T),
                                           pk(eS, kb, qb, T))

                        for kt in range(4):  # one psum/exp group per kt
                            pss = psS.tile([P, 2 * TQ], f32, tag="psS",
                                           name="psS")
                            nc.tensor.matmul(
                                pss[:, kt * P:TQ],
                                kT[h][:, kt * P:(kt + 1) * P],
                                qT[h][:, kt * P:TQ],
                                start=True, stop=True)
                            nc.tensor.matmul(
                                pss[:, TQ:2 * TQ],
                                kT[h][:, kt * P:(kt + 1) * P],
                                qT[h][:, TQ:2 * TQ],
                                start=True, stop=True)
                            nc.scalar.activation(
                                out=pk(eS, kt, kt * P, T),
                                in_=pss[:, kt * P:2 * TQ], func=AFT.Exp)
                            mask(kt)
                            if kt % 2 == 1:
                                pair_add(kt // 2)
                            pv_mms([kt])
                        # g4: kt4 (full) + kt5 (384 cols) in one 2-bank psum
                        pss = psS.tile([P, 2 * TQ], f32, tag="psS", name="psS")
                        nc.tensor.matmul(pss[:, 0:TQ],
                                         kT[h][:, 4 * P:5 * P],
                                         qT[h][:, TQ:2 * TQ],
                                         start=True, stop=True)
                        nc.tensor.matmul(pss[:, TQ:TQ + 384],
                                         kT[h][:, 5 * P:6 * P],
                                         qT[h][:, TQ + P:2 * TQ],
                                         start=True, stop=True)
                        nc.scalar.activation(
                            out=eS[:, SROW[4]:SROW[4] + 896],
                            in_=pss[:, 0:896], func=AFT.Exp)
                        mask(4)
                        mask(5)
                        pair_add(2)
                        pv_mms([4, 5])
                        # g5: kt6 (256) + kt7 (128) packed into one bank
                        pss = psS.tile([P, 2 * TQ], f32, tag="psS", name="psS")
                        nc.tensor.matmul(pss[:, 0:256],
                                         kT[h][:, 6 * P:7 * P],
                                         qT[h][:, TQ + 2 * P:2 * TQ],
                                         start=True, stop=False)
                        nc.tensor.matmul(pss[:, 256:384],
                                         kT[h][:, 7 * P:8 * P],
                                         qT[h][:, TQ + 3 * P:2 * TQ],
                                         start=False, stop=True)
                        nc.scalar.activation(
                            out=eS[:, SROW[6]:ES_W],
                            in_=pss[:, 0:384], func=AFT.Exp)
                        mask(6)
                        mask(7)
                        pair_add(3)
                        pv_mms([6, 7])
                        return eS, eP, pso

                    def c_denom_div(h, eP, pso):
                        """Denominators from pair-sums (narrowed to each
                        pair's valid queries), then divide."""
                        psd = [psD.tile([P, TQ], f32, tag="psD", name="psD")
                               for _ in range(NQ)]
                        # pair i is valid for q >= 2i*128
                        for qt in range(NQ):
                            lastp = 1 if qt == 0 else NPAIR - 1
                            for i in range(lastp + 1):
                                q0 = max(qt * TQ, 2 * i * P)
                                q1 = (qt + 1) * TQ
                                nc.tensor.matmul(
                                    psd[qt][:, q0 - qt * TQ:TQ], ones_sb,
                                    ep(eP, i, q0, q1),
                                    start=(i == 0), stop=(i == lastp))
                        for qt in range(NQ):
                            rec = ctmp.tile([P, TQ], f32, tag="rec", name="rec")
                            # ~18-bit accurate, 5x faster than reciprocal();
                            # denominators are in [1, ~2e5] so edge cases are
                            # impossible
                            nc.vector.reciprocal_approx_fast(out=rec,
                                                             in_=psd[qt])
                            nc.vector.tensor_mul(
                                attnT[h][:, qt * TQ:(qt + 1) * TQ],
                                pso[:, qt, :], rec)

                    # Software-pipelined: PV runs one head behind the scores
                    # (interleaved between score groups), denominators two
                    # heads behind. The denom/div of head h-2 is emitted first
                    # so its divide frees the single PV PSUM slot before the
                    # PV matmuls of head h-1 need it.
                    st = {}
                    for h in range(H):
                        if h >= 2:
                            c_denom_div(h - 2, st[h - 2][1], st[h - 1][2])
                        prev_eS = st[h - 1][0] if h >= 1 else None
                        st[h] = c_front(h, h - 1 if h >= 1 else None, prev_eS)
                    c_denom_div(H - 2, st[H - 2][1], st[H - 1][2])
                    # final head's PV + denom/div
                    pso_last = psO.tile([P, NQ, TQ], f32, tag="psO", name="psO")
                    eS_last = st[H - 1][0]
                    for kt in range(TT):
                        for qt in range(NQ):
                            q0 = max(qt * TQ, kt * P)
                            q1 = (qt + 1) * TQ
                            if q0 >= q1:
                                continue
                            nc.tensor.matmul(
                                pso_last[:, qt, q0 - qt * TQ:TQ],
                                vv[:, kt, (H - 1) * P:H * P],
                                pk(eS_last, kt, q0, q1),
                                start=(kt == 0), stop=(kt == min(7, qt * 4 + 3)))
                    c_denom_div(H - 1, st[H - 1][1], pso_last)

                # ---- Phase D: output projection (ct-outer, t-half groups) ----
                with tc.tile_pool(name="ybuf", bufs=4) as ybuf, \
                     tc.tile_pool(name="psY", bufs=8, space=bass.MemorySpace.PSUM) as psYp:
                    NCT = C // TQ  # 4

                    wp = wp0
                    for ct in range(NCT):
                        wp_next = d_load(ct + 1) if ct + 1 < NCT else None
                        for half in range(2):
                            ts = range(half * 4, half * 4 + 4)
                            psY = {t: psYp.tile([P, TQ], f32, tag="psY",
                                                name="psY") for t in ts}
                            for kk in range(KK):
                                for t in ts:
                                    nc.tensor.matmul(
                                        psY[t], attnT[kk][:, t * P:(t + 1) * P],
                                        wp[:, kk, :],
                                        start=(kk == 0), stop=(kk == KK - 1))
                            for t in ts:
                                y_sb = ybuf.tile([P, TQ], f32, tag="y_sb",
                                                 name="y_sb")
                                nc.vector.tensor_add(
                                    y_sb, psY[t],
                                    bias_sb[:, ct * TQ:(ct + 1) * TQ])
                                # y stores alternate between the gpsimd and
                                # scalar DMA queues (parallel to the w_proj
                                # loads on the sync queue)
                                eng = nc.gpsimd if t % 2 == 0 else nc.scalar
                                eng.dma_start(
                                    out=y[t * P:(t + 1) * P,
                                          ct * TQ:(ct + 1) * TQ],
                                    in_=y_sb)
                        wp = wp_next

    nc.compile()
    return nc


def _get_nc():
    global _NC_CACHE
    if _NC_CACHE is None:
        _NC_CACHE = build_nc()
    return _NC_CACHE


def make_in_maps(inputs):
    x = np.asarray(inputs["x"], dtype=np.float32)
    w_attn = np.asarray(inputs["w_attn"], dtype=np.float32)
    b_attn = np.asarray(inputs["b_attn"], dtype=np.float32)
    w_proj = np.asarray(inputs["w_proj"], dtype=np.float32)
    b_proj = np.asarray(inputs["b_proj"], dtype=np.float32)

    # cast to bf16 host-side (device computes in bf16 anyway) and repack
    # weights so each SBUF partition reads one contiguous block
    MCH = 3 * C // P
    x_bf = np.ascontiguousarray(x.astype(ml_dtypes.bfloat16))
    w_attn_pk = np.ascontiguousarray(
        w_attn.astype(ml_dtypes.bfloat16)
        .reshape(KK, P, MCH, P).transpose(1, 2, 0, 3))
    w_proj_pk = np.ascontiguousarray(
        w_proj.astype(ml_dtypes.bfloat16)
        .reshape(KK, P, C // TQ, TQ).transpose(1, 2, 0, 3))

    # bias prep: [3C] -> [P, 48] partition-major; q columns folded with scale
    bpm = np.ascontiguousarray(b_attn.reshape(3 * C // P, P).T).copy()
    bpm[:, :16] *= SCALE
    bias_bc = np.ascontiguousarray(
        np.broadcast_to(b_proj.reshape(1, C), (P, C)).astype(np.float32))

    # single [128,128] causal triangle (valid where q_local >= k_local)
    masks = (np.arange(P)[None, :] >= np.arange(P)[:, None]).astype(
        ml_dtypes.bfloat16)
    ident_b = np.eye(P, dtype=ml_dtypes.bfloat16)
    ones_b = np.ones((P, P), dtype=ml_dtypes.bfloat16)

    common = dict(w_attn_pk=w_attn_pk, b_attn_pm=bpm, w_proj_pk=w_proj_pk,
                  bias_bc=bias_bc, masks=masks,
                  ident_b=ident_b, ones_b=ones_b)
    return [dict(x_bf=np.ascontiguousarray(x_bf[i]), **common)
            for i in range(B)]


def run_spmd(inputs, trace=False, **kw):
    nc = _get_nc()
    in_maps = make_in_maps(inputs)
    return run_bass_kernel_spmd(nc, in_maps, list(range(N_CORES)),
                                trace=trace, **kw)


def kernel(**inputs):
    res = run_spmd(inputs, trace=False)
    y = np.stack([np.asarray(res.results[i]["y"]) for i in range(N_CORES)])
    return y.astype(np.float32)


if __name__ == "__main__":
    rng = np.random.default_rng(0)
    demo = {
        "x": rng.standard_normal((B, T, C)).astype(np.float32),
        "w_attn": (rng.standard_normal((C, 3 * C)) * 0.02).astype(np.float32),
        "b_attn": (rng.standard_normal(3 * C) * 0.02).astype(np.float32),
        "w_proj": (rng.standard_normal((C, C)) * 0.02).astype(np.float32),
        "b_proj": (rng.standard_normal(C) * 0.02).astype(np.float32),
    }
    out = kernel(**demo)
    print("out", out.shape, out.dtype, float(np.abs(out).max()))
